# revision 11
# baseline (speedup 1.0000x reference)
"""MoE (LlamaSparseMoeBlock) Trainium2 kernel — expert-parallel over 8 NeuronCores.

Strategy:
  - Core c owns expert c (E == n_cores == 8).
  - Each core computes router logits for its 1/8 token slice (fp32 matmul),
    AllGathers the logits so every core has identical fp32 routing decisions.
  - Each core computes top-2 + normalized weights for ALL tokens, compacts the
    token ids routed to ITS expert (PE prefix-sum matmul + indirect-DMA
    scatter/gather of an index table), gathers those token rows (bf16),
    runs the FFN (bf16 matmuls, fp32 accumulate), scales rows by the routing
    weight, scatters the fp16 rows into a zeroed [T, H] partial buffer, and a
    ReduceScatter(add) combines partials; core c ends with output rows
    [c*1024, (c+1)*1024), cast to fp32.
"""

import numpy as np
import ml_dtypes

import concourse.bass as bass
import concourse.mybir as mybir
from concourse import bacc, bass_utils
from concourse.bass import ts, IndirectOffsetOnAxis
from concourse.tile import TileContext
from concourse.masks import make_identity, make_upper_triangular
from concourse.tile_rust import add_dep_helper

F32 = mybir.dt.float32
F16 = mybir.dt.float16
BF16 = mybir.dt.bfloat16
I32 = mybir.dt.int32
U32 = mybir.dt.uint32

E = 8
K = 2
H = 1024
F = 2048
T = 8192           # total tokens (2*4096)
TPC = T // 8       # tokens per core = 1024
CAP = 2560         # max tokens routed to one expert (20 tiles of 128)
GT = CAP // 128    # gather tiles = 20
NMEGA = GT // 4    # FFN megatiles of 512 tokens = 5
KO_H = H // 128    # 8
KO_F = F // 128    # 16
TI = TPC // 128    # 8 token tiles per core slice
TI_ALL = T // 128  # 64 token tiles globally
RG = [[0, 1, 2, 3, 4, 5, 6, 7]]
OOB = 1_000_000_000

_cached = {}


def _build():
    nc = bacc.Bacc("TRN2", num_devices=8)

    # ---- I/O ----
    xT_c = nc.dram_tensor("xT_c", [H, TPC], F32, kind="ExternalInput")
    x_bf = nc.dram_tensor("x_bf", [T, H], BF16, kind="ExternalInput")
    gwt = nc.dram_tensor("gwt", [H, E], F32, kind="ExternalInput")
    wg = nc.dram_tensor("wg", [H, F], BF16, kind="ExternalInput")
    wu = nc.dram_tensor("wu", [H, F], BF16, kind="ExternalInput")
    wd = nc.dram_tensor("wd", [F, H], BF16, kind="ExternalInput")
    out_sl = nc.dram_tensor("out_sl", [TPC, H], F32, kind="ExternalOutput")
    log_sl = nc.dram_tensor("log_sl", [TPC, E], F32, kind="ExternalOutput")

    # ---- internal DRAM ----
    ag_in = nc.dram_tensor("ag_in", [TPC, E], F32, kind="Internal")
    ag_out = nc.dram_tensor("ag_out", [T, E], F32, kind="Internal", addr_space="Shared")
    idx_dram = nc.dram_tensor("idx_dram", [CAP, 4], I32, kind="Internal")
    partial = nc.dram_tensor("partial", [T, H], F16, kind="Internal")
    rs_out = nc.dram_tensor("rs_out", [TPC, H], F16, kind="Internal")

    with TileContext(nc) as tc:
        with (
            tc.tile_pool(name="wpool", bufs=1) as wpool,
            tc.tile_pool(name="cpool", bufs=1) as cpool,
            tc.tile_pool(name="spool", bufs=2) as spool,
            tc.tile_pool(name="gpool", bufs=2) as gpool,
            tc.tile_pool(name="psA", bufs=2, space="PSUM") as psA,
            tc.tile_pool(name="psB", bufs=2, space="PSUM") as psB,
            tc.tile_pool(name="psT", bufs=2, space="PSUM") as psT,
            tc.tile_pool(name="psO", bufs=1, space="PSUM") as psO,
        ):
            # ======== phase 0: constants + preloads ========
            id_bf = cpool.tile([128, 128], BF16, tag="id_bf")
            make_identity(nc, id_bf[:])
            ustrict = cpool.tile([128, 128], F32, tag="ustrict")
            make_upper_triangular(nc, ustrict[:], val=1.0, diag=False)

            iota8_i = cpool.tile([128, E], I32, tag="iota8i")
            nc.gpsimd.iota(iota8_i[:], pattern=[[1, E]], base=0, channel_multiplier=0)
            iota8f = cpool.tile([128, E], F32, tag="iota8f")
            nc.vector.tensor_copy(iota8f[:], iota8_i[:])
            iota8m = cpool.tile([128, E], F32, tag="iota8m")  # iota - 8
            nc.vector.tensor_scalar_sub(iota8m[:], iota8f[:], 8.0)

            tokid_i = cpool.tile([128, TI_ALL], I32, tag="tokidi")
            nc.gpsimd.iota(tokid_i[:], pattern=[[128, TI_ALL]], base=0, channel_multiplier=1)

            # partition id -> fp32 broadcast [128, 1]
            pid_u = cpool.tile([1, 1], U32, tag="pidu")
            assert nc.partition_id_tensor is not None
            nc.sync.dma_start(pid_u[:], nc.partition_id_tensor[:])
            pid_f1 = cpool.tile([1, 1], F32, tag="pidf1")
            nc.vector.tensor_copy(pid_f1[:], pid_u[:])
            pidb = cpool.tile([128, 1], F32, tag="pidb")
            nc.gpsimd.partition_broadcast(pidb[:], pid_f1[:])

            # weights -> SBUF (bf16)
            wg_sb = wpool.tile([128, KO_H, F], BF16, tag="wg")
            nc.sync.dma_start(wg_sb[:], wg[:].rearrange("(ko p) f -> p ko f", p=128))
            wu_sb = wpool.tile([128, KO_H, F], BF16, tag="wu")
            nc.sync.dma_start(wu_sb[:], wu[:].rearrange("(ko p) f -> p ko f", p=128))
            wd_sb = wpool.tile([128, KO_F, H], BF16, tag="wd")
            nc.sync.dma_start(wd_sb[:], wd[:].rearrange("(ko p) h -> p ko h", p=128))

            # router inputs
            xT_r = xT_c[:].rearrange("(ko p) t -> p ko t", p=128)
            gwt_sb = cpool.tile([128, KO_H, E], F32, tag="gwt")
            nc.sync.dma_start(gwt_sb[:], gwt[:].rearrange("(ko p) e -> p ko e", p=128))

            # zero the fp16 partial buffer [T, H]
            zero_sb = cpool.tile([128, H], F16, tag="zero")
            nc.vector.memset(zero_sb[:], 0.0)
            part_r = partial[:].rearrange("(t p) h -> p t h", p=128)
            zero_dmas = []
            for j in range(TI_ALL):
                d = nc.sync.dma_start(part_r[:, j, :], zero_sb[:])
                zero_dmas.append(d)

            # init idx_dram with OOB sentinels
            oob_sb = cpool.tile([128, GT, 4], I32, tag="oob")
            nc.vector.memset(oob_sb[:], OOB)
            idx_r = idx_dram[:].rearrange("(g p) c -> p g c", p=128)
            init_idx = nc.sync.dma_start(idx_r, oob_sb[:])

            # ======== phase 1: router (fp32) ========
            ag_writes = []
            for tt in range(TI):
                ps_l_full = psA.tile([128, 512], F32, tag="psA", name=f"ps_l{tt}")
                ps_l = ps_l_full[:, :E]
                for k in range(KO_H):
                    xrot = spool.tile([128, 128], F32, tag="xrot", name=f"xr{tt}_{k}")
                    nc.sync.dma_start(xrot[:], xT_r[:, k, ts(tt, 128)])
                    nc.tensor.matmul(
                        ps_l,
                        lhsT=xrot[:],
                        rhs=gwt_sb[:, k, :],
                        start=(k == 0),
                        stop=(k == KO_H - 1),
                    )
                lg_sb = spool.tile([128, E], F32, tag="lgsb")
                nc.scalar.copy(lg_sb[:], ps_l)
                d1 = nc.sync.dma_start(ag_in[ts(tt, 128), :], lg_sb[:])
                nc.sync.dma_start(log_sl[ts(tt, 128), :], lg_sb[:])
                ag_writes.append(d1)

            # ======== phase 2: AllGather of logits ========
            ag = nc.gpsimd.collective_compute(
                kind="AllGather",
                op=mybir.AluOpType.bypass,
                replica_groups=RG,
                ins=[ag_in[:]],
                outs=[ag_out[:]],
            )
            for d in ag_writes:
                add_dep_helper(ag.ins, d.ins, reason="ag after logit writes")

            # ======== phase 3: top-2 + weights + compaction (all T tokens) ========
            rt = cpool.tile([128, TI_ALL, E], F32, tag="rt")
            rt_load = nc.sync.dma_start(
                rt[:], ag_out[:].rearrange("(ti p) e -> p ti e", p=128)
            )
            add_dep_helper(rt_load.ins, ag.ins, reason="rt load after AG")

            max1 = cpool.tile([128, TI_ALL], F32, tag="max1")
            nc.vector.tensor_reduce(max1[:], rt[:], axis=mybir.AxisListType.X,
                                    op=mybir.AluOpType.max)
            eq1 = spool.tile([128, TI_ALL, E], F32, tag="scr8")
            nc.vector.tensor_tensor(eq1[:], rt[:],
                                    max1[:, :, None].to_broadcast([128, TI_ALL, E]),
                                    mybir.AluOpType.is_equal)
            cand = spool.tile([128, TI_ALL, E], F32, tag="scr8")
            nc.vector.tensor_tensor(cand[:], eq1[:],
                                    iota8m[:, None, :].to_broadcast([128, TI_ALL, E]),
                                    mybir.AluOpType.mult)
            nc.vector.tensor_scalar_add(cand[:], cand[:], 8.0)
            idx1 = cpool.tile([128, TI_ALL], F32, tag="idx1")
            nc.vector.tensor_reduce(idx1[:], cand[:], axis=mybir.AxisListType.X,
                                    op=mybir.AluOpType.min)

            # mask out idx1 position by index, find second max
            nmask = spool.tile([128, TI_ALL, E], F32, tag="scr8")
            nc.vector.tensor_tensor(nmask[:],
                                    iota8f[:, None, :].to_broadcast([128, TI_ALL, E]),
                                    idx1[:, :, None].to_broadcast([128, TI_ALL, E]),
                                    mybir.AluOpType.is_equal)
            nc.vector.tensor_scalar_mul(nmask[:], nmask[:], -1e30)
            rt2 = cpool.tile([128, TI_ALL, E], F32, tag="rt2")
            nc.vector.tensor_tensor(rt2[:], rt[:], nmask[:], mybir.AluOpType.add)
            max2 = cpool.tile([128, TI_ALL], F32, tag="max2")
            nc.vector.tensor_reduce(max2[:], rt2[:], axis=mybir.AxisListType.X,
                                    op=mybir.AluOpType.max)
            eq2 = spool.tile([128, TI_ALL, E], F32, tag="scr8")
            nc.vector.tensor_tensor(eq2[:], rt2[:],
                                    max2[:, :, None].to_broadcast([128, TI_ALL, E]),
                                    mybir.AluOpType.is_equal)
            cand2 = spool.tile([128, TI_ALL, E], F32, tag="scr8")
            nc.vector.tensor_tensor(cand2[:], eq2[:],
                                    iota8m[:, None, :].to_broadcast([128, TI_ALL, E]),
                                    mybir.AluOpType.mult)
            nc.vector.tensor_scalar_add(cand2[:], cand2[:], 8.0)
            idx2 = cpool.tile([128, TI_ALL], F32, tag="idx2")
            nc.vector.tensor_reduce(idx2[:], cand2[:], axis=mybir.AxisListType.X,
                                    op=mybir.AluOpType.min)

            # w1 = sigmoid(max1 - max2); my-expert mask + weight
            dmx = cpool.tile([128, TI_ALL], F32, tag="dmx")
            nc.vector.tensor_tensor(dmx[:], max1[:], max2[:], mybir.AluOpType.subtract)
            w1 = cpool.tile([128, TI_ALL], F32, tag="w1")
            nc.scalar.activation(w1[:], dmx[:], mybir.ActivationFunctionType.Sigmoid)

            sel1 = cpool.tile([128, TI_ALL], F32, tag="sel1")
            nc.vector.tensor_tensor(sel1[:], idx1[:],
                                    pidb[:].to_broadcast([128, TI_ALL]),
                                    mybir.AluOpType.is_equal)
            sel2 = cpool.tile([128, TI_ALL], F32, tag="sel2")
            nc.vector.tensor_tensor(sel2[:], idx2[:],
                                    pidb[:].to_broadcast([128, TI_ALL]),
                                    mybir.AluOpType.is_equal)
            m_e = cpool.tile([128, TI_ALL], F32, tag="me")
            nc.vector.tensor_tensor(m_e[:], sel1[:], sel2[:], mybir.AluOpType.add)
            # w_e = sel2 + w1*(sel1-sel2)
            w_e = cpool.tile([128, TI_ALL], F32, tag="we")
            nc.vector.tensor_tensor(w_e[:], sel1[:], sel2[:], mybir.AluOpType.subtract)
            nc.vector.tensor_tensor(w_e[:], w_e[:], w1[:], mybir.AluOpType.mult)
            nc.vector.tensor_tensor(w_e[:], w_e[:], sel2[:], mybir.AluOpType.add)

            # exclusive cumsum of m_e along tokens within each 128-tile (PE)
            ps_c_full = psA.tile([128, 512], F32, tag="psA", name="ps_c")
            ps_c = ps_c_full[:, :TI_ALL]
            nc.tensor.matmul(ps_c, lhsT=ustrict[:], rhs=m_e[:], start=True, stop=True)
            pos = cpool.tile([128, TI_ALL], F32, tag="pos")
            nc.vector.tensor_copy(pos[:], ps_c)

            # per-tile totals S[1, 64] via ones-vector matmul; then exclusive
            # cumsum across tiles (log-shift)
            ones_c = cpool.tile([128, 1], F32, tag="onesc")
            nc.vector.memset(ones_c[:], 1.0)
            ps_s_full = psB.tile([128, 512], F32, tag="psB", name="ps_s")
            ps_s = ps_s_full[:1, :TI_ALL]
            nc.tensor.matmul(ps_s, lhsT=ones_c[:], rhs=m_e[:], start=True, stop=True)
            s_tot = cpool.tile([1, TI_ALL], F32, tag="stot")
            nc.vector.tensor_copy(s_tot[:], ps_s)
            a_t = cpool.tile([1, TI_ALL], F32, tag="csA")
            b_t = cpool.tile([1, TI_ALL], F32, tag="csB")
            nc.vector.tensor_copy(a_t[:], s_tot[:])
            cur, nxt = a_t, b_t
            for sh in [1, 2, 4, 8, 16, 32]:
                nc.vector.tensor_copy(nxt[:, :sh], cur[:, :sh])
                nc.vector.tensor_tensor(nxt[:, sh:], cur[:, sh:], cur[:, : TI_ALL - sh],
                                        mybir.AluOpType.add)
                cur, nxt = nxt, cur
            base_1 = cpool.tile([1, TI_ALL], F32, tag="base1")  # exclusive = incl - S
            nc.vector.tensor_tensor(base_1[:], cur[:], s_tot[:], mybir.AluOpType.subtract)
            base_b = cpool.tile([128, TI_ALL], F32, tag="baseb")
            nc.gpsimd.partition_broadcast(base_b[:], base_1[:])

            dest = cpool.tile([128, TI_ALL], F32, tag="dest")
            nc.vector.tensor_tensor(dest[:], pos[:], base_b[:], mybir.AluOpType.add)
            # unselected tokens -> OOB
            gate_oob = cpool.tile([128, TI_ALL], F32, tag="goob")
            nc.vector.tensor_scalar(gate_oob[:], m_e[:], -1.0e9, 1.0e9,
                                    op0=mybir.AluOpType.mult, op1=mybir.AluOpType.add)
            nc.vector.tensor_tensor(dest[:], dest[:], gate_oob[:], mybir.AluOpType.add)
            dest_i = cpool.tile([128, TI_ALL], I32, tag="desti")
            nc.vector.tensor_copy(dest_i[:], dest[:])

            # payload rows (token_id, w_bits, 0, 0) scattered to idx_dram[dest]
            payload = cpool.tile([128, TI_ALL, 4], I32, tag="payload")
            nc.vector.memset(payload[:], 0)
            nc.vector.tensor_copy(payload[:, :, 0:1], tokid_i[:, :, None])
            nc.vector.tensor_copy(payload[:, :, 1:2].bitcast(F32), w_e[:, :, None])
            scatters = []
            for ti in range(TI_ALL):
                sc = nc.gpsimd.indirect_dma_start(
                    out=idx_dram[:],
                    out_offset=IndirectOffsetOnAxis(ap=dest_i[:, ti : ti + 1], axis=0),
                    in_=payload[:, ti, :],
                    in_offset=None,
                    bounds_check=CAP - 1,
                    oob_is_err=False,
                )
                add_dep_helper(sc.ins, init_idx.ins, reason="scatter after idx init")
                scatters.append(sc)

            # load back the compacted (token_id, weight) table
            idx_sb = cpool.tile([128, GT, 4], I32, tag="idxsb")
            idx_load = nc.sync.dma_start(idx_sb[:], idx_r)
            for sc in scatters:
                add_dep_helper(idx_load.ins, sc.ins, reason="idx load after scatters")

            # ======== phase 4+5: gather + FFN + weighted scatter ========
            rs_deps = list(zero_dmas)
            for m in range(NMEGA):
                xg = gpool.tile([128, 4, H], BF16, tag="xg")
                for j in range(4):
                    g = 4 * m + j
                    nc.gpsimd.indirect_dma_start(
                        out=xg[:, j, :],
                        out_offset=None,
                        in_=x_bf[:],
                        in_offset=IndirectOffsetOnAxis(ap=idx_sb[:, g, 0:1], axis=0),
                        bounds_check=T - 1,
                        oob_is_err=False,
                    )
                # transpose to [H(part), 512]
                xgt = gpool.tile([128, KO_H, 512], BF16, tag="xgt")
                for j in range(4):
                    for kt in range(KO_H):
                        ps_t = psT.tile([128, 128], BF16, tag="psT")
                        nc.tensor.transpose(ps_t[:], xg[:, j, ts(kt, 128)], id_bf[:])
                        nc.scalar.copy(xgt[:, kt, ts(j, 128)], ps_t[:])

                # m1/m2: gT[f, tok] = silu(Wg.T x) * (Wu.T x)
                gt_sb = wpool.tile([128, KO_F, 512], BF16, tag="gt")
                for fb in range(KO_F):
                    ps_g = psA.tile([128, 512], F32, tag="psA")
                    ps_u = psB.tile([128, 512], F32, tag="psB")
                    for k in range(KO_H):
                        nc.tensor.matmul(ps_g, lhsT=wg_sb[:, k, ts(fb, 128)],
                                         rhs=xgt[:, k, :],
                                         start=(k == 0), stop=(k == KO_H - 1))
                    for k in range(KO_H):
                        nc.tensor.matmul(ps_u, lhsT=wu_sb[:, k, ts(fb, 128)],
                                         rhs=xgt[:, k, :],
                                         start=(k == 0), stop=(k == KO_H - 1))
                    sil = spool.tile([128, 512], F32, tag="sil")
                    nc.scalar.activation(sil[:], ps_g, mybir.ActivationFunctionType.Silu)
                    nc.vector.tensor_tensor(gt_sb[:, fb, :], sil[:], ps_u,
                                            mybir.AluOpType.mult)

                # m3: out[tok, H] = gT.T @ Wd ; scale by w; scatter to partial
                for tb in range(4):
                    g = 4 * m + tb
                    ps_o = psO.tile([128, H], F32, tag="psO")
                    for fs in range(KO_F):
                        nc.tensor.matmul(ps_o[:, :512],
                                         lhsT=gt_sb[:, fs, ts(tb, 128)],
                                         rhs=wd_sb[:, fs, :512],
                                         start=(fs == 0), stop=(fs == KO_F - 1))
                    for fs in range(KO_F):
                        nc.tensor.matmul(ps_o[:, 512:],
                                         lhsT=gt_sb[:, fs, ts(tb, 128)],
                                         rhs=wd_sb[:, fs, 512:],
                                         start=(fs == 0), stop=(fs == KO_F - 1))
                    outw = spool.tile([128, H], F16, tag="outw")
                    wcol = idx_sb[:, g, 1:2].bitcast(F32)
                    nc.vector.tensor_tensor(outw[:], ps_o[:],
                                            wcol.to_broadcast([128, H]),
                                            mybir.AluOpType.mult)
                    sc = nc.gpsimd.indirect_dma_start(
                        out=partial[:],
                        out_offset=IndirectOffsetOnAxis(ap=idx_sb[:, g, 0:1], axis=0),
                        in_=outw[:],
                        in_offset=None,
                        bounds_check=T - 1,
                        oob_is_err=False,
                    )
                    rs_deps.append(sc)

            # ======== phase 6: ReduceScatter(add) ========
            rs = nc.gpsimd.collective_compute(
                kind="ReduceScatter",
                op=mybir.AluOpType.add,
                replica_groups=RG,
                ins=[partial[:]],
                outs=[rs_out[:]],
            )
            for d in rs_deps:
                add_dep_helper(rs.ins, d.ins, reason="rs after partial writes")

            # ======== phase 7: cast fp16 -> fp32 output slice ========
            rs_r = rs_out[:].rearrange("(t p) h -> p t h", p=128)
            out_r = out_sl[:].rearrange("(t p) h -> p t h", p=128)
            for tt in range(TI):
                h16 = spool.tile([128, H], F16, tag="h16")
                ld = nc.sync.dma_start(h16[:], rs_r[:, tt, :])
                add_dep_helper(ld.ins, rs.ins, reason="read rs_out after RS")
                h32 = spool.tile([128, H], F32, tag="h32")
                nc.vector.tensor_copy(h32[:], h16[:])
                nc.sync.dma_start(out_r[:, tt, :], h32[:])

    nc.finalize()
    return nc


def _get_nc():
    if "nc" not in _cached:
        _cached["nc"] = _build()
    return _cached["nc"]


def kernel(hidden_states, gate_w, Wg, Wu, Wd, _trace=False):
    nc = _get_nc()
    b, s, h = hidden_states.shape
    x2d = np.ascontiguousarray(np.asarray(hidden_states, dtype=np.float32).reshape(-1, h))
    gate_w = np.asarray(gate_w, dtype=np.float32)
    x_bf = np.ascontiguousarray(x2d.astype(ml_dtypes.bfloat16))
    gwt = np.ascontiguousarray(gate_w.T)
    Wg = np.asarray(Wg, dtype=np.float32)
    Wu = np.asarray(Wu, dtype=np.float32)
    Wd = np.asarray(Wd, dtype=np.float32)

    in_maps = []
    for c in range(8):
        in_maps.append({
            "xT_c": np.ascontiguousarray(x2d[c * TPC : (c + 1) * TPC].T),
            "x_bf": x_bf,
            "gwt": gwt,
            "wg": np.ascontiguousarray(Wg[c]).astype(ml_dtypes.bfloat16),
            "wu": np.ascontiguousarray(Wu[c]).astype(ml_dtypes.bfloat16),
            "wd": np.ascontiguousarray(Wd[c]).astype(ml_dtypes.bfloat16),
        })

    res = bass_utils.run_bass_kernel_spmd(
        nc, in_maps, core_ids=list(range(8)), trace=_trace
    )
    _cached["last_res"] = res
    out = np.concatenate([r["out_sl"] for r in res.results], axis=0)
    logits = np.concatenate([r["log_sl"] for r in res.results], axis=0)
    return out.reshape(b, s, h).astype(np.float32), logits.astype(np.float32)


# revision 14
# speedup vs baseline: 1.1436x; 1.1436x over previous
"""MoE (LlamaSparseMoeBlock) Trainium2 kernel — expert-parallel over 8 NeuronCores.

Strategy:
  - Core c owns expert c (E == n_cores == 8).
  - Each core computes router logits for its 1/8 token slice (fp32 matmul),
    AllGathers the logits so every core has identical fp32 routing decisions.
  - Each core computes top-2 + normalized weights for ALL tokens, compacts the
    token ids routed to ITS expert (PE prefix-sum matmul + indirect-DMA
    scatter/gather of an index table), gathers those token rows (bf16),
    runs the FFN (bf16 matmuls, fp32 accumulate), scales rows by the routing
    weight, scatters the fp16 rows into a zeroed [T, H] partial buffer, and a
    ReduceScatter(add) combines partials; core c ends with output rows
    [c*1024, (c+1)*1024), cast to fp32.
"""

import numpy as np
import ml_dtypes

import concourse.bass as bass
import concourse.mybir as mybir
from concourse import bacc, bass_utils
from concourse.bass import ts, IndirectOffsetOnAxis
from concourse.tile import TileContext
from concourse.masks import make_identity, make_upper_triangular
from concourse.tile_rust import add_dep_helper

F32 = mybir.dt.float32
F16 = mybir.dt.float16
BF16 = mybir.dt.bfloat16
I32 = mybir.dt.int32
U32 = mybir.dt.uint32

E = 8
K = 2
H = 1024
F = 2048
T = 8192           # total tokens (2*4096)
TPC = T // 8       # tokens per core = 1024
CAP = 2560         # max tokens routed to one expert (20 tiles of 128)
GT = CAP // 128    # gather tiles = 20
NMEGA = GT // 4    # FFN megatiles of 512 tokens = 5
KO_H = H // 128    # 8
KO_F = F // 128    # 16
TI = TPC // 128    # 8 token tiles per core slice
TI_ALL = T // 128  # 64 token tiles globally
RG = [[0, 1, 2, 3, 4, 5, 6, 7]]
OOB = 1_000_000_000

_cached = {}


def _build():
    nc = bacc.Bacc("TRN2", num_devices=8)

    # ---- I/O ----
    xT_c = nc.dram_tensor("xT_c", [H, TPC], F32, kind="ExternalInput")
    x_bf = nc.dram_tensor("x_bf", [T, H], BF16, kind="ExternalInput")
    gwt = nc.dram_tensor("gwt", [H, E], F32, kind="ExternalInput")
    wg = nc.dram_tensor("wg", [H, F], BF16, kind="ExternalInput")
    wu = nc.dram_tensor("wu", [H, F], BF16, kind="ExternalInput")
    wd = nc.dram_tensor("wd", [F, H], BF16, kind="ExternalInput")
    out_sl = nc.dram_tensor("out_sl", [TPC, H], F32, kind="ExternalOutput")
    log_sl = nc.dram_tensor("log_sl", [TPC, E], F32, kind="ExternalOutput")

    # ---- internal DRAM ----
    ag_in = nc.dram_tensor("ag_in", [TPC, E], F32, kind="Internal")
    ag_out = nc.dram_tensor("ag_out", [T, E], F32, kind="Internal", addr_space="Shared")
    idx_dram = nc.dram_tensor("idx_dram", [CAP, 4], I32, kind="Internal")
    partial = nc.dram_tensor("partial", [T, H], F16, kind="Internal")
    rs_out = nc.dram_tensor("rs_out", [TPC, H], F16, kind="Internal")

    with TileContext(nc) as tc:
        with (
            tc.tile_pool(name="wpool", bufs=1) as wpool,
            tc.tile_pool(name="cpool", bufs=1) as cpool,
            tc.tile_pool(name="spool", bufs=2) as spool,
            tc.tile_pool(name="gpool", bufs=2) as gpool,
            tc.tile_pool(name="psA", bufs=1, space="PSUM") as psA,
            tc.tile_pool(name="psB", bufs=1, space="PSUM") as psB,
            tc.tile_pool(name="psT", bufs=2, space="PSUM") as psT,
            tc.tile_pool(name="psO", bufs=2, space="PSUM") as psO,
        ):
            # ======== phase 0: constants + preloads ========
            id_bf = cpool.tile([128, 128], BF16, tag="id_bf")
            make_identity(nc, id_bf[:])
            ustrict = cpool.tile([128, 128], F32, tag="ustrict")
            make_upper_triangular(nc, ustrict[:], val=1.0, diag=False)

            iota8_i = cpool.tile([128, E], I32, tag="iota8i")
            nc.gpsimd.iota(iota8_i[:], pattern=[[1, E]], base=0, channel_multiplier=0)
            iota8f = cpool.tile([128, E], F32, tag="iota8f")
            nc.vector.tensor_copy(iota8f[:], iota8_i[:])
            iota8m = cpool.tile([128, E], F32, tag="iota8m")  # iota - 8
            nc.vector.tensor_scalar_sub(iota8m[:], iota8f[:], 8.0)

            tokid_i = cpool.tile([128, TI_ALL], I32, tag="tokidi")
            nc.gpsimd.iota(tokid_i[:], pattern=[[128, TI_ALL]], base=0, channel_multiplier=1)

            # partition id -> fp32 broadcast [128, 1]
            pid_u = cpool.tile([1, 1], U32, tag="pidu")
            assert nc.partition_id_tensor is not None
            nc.sync.dma_start(pid_u[:], nc.partition_id_tensor[:])
            pid_f1 = cpool.tile([1, 1], F32, tag="pidf1")
            nc.vector.tensor_copy(pid_f1[:], pid_u[:])
            pidb = cpool.tile([128, 1], F32, tag="pidb")
            nc.gpsimd.partition_broadcast(pidb[:], pid_f1[:])

            # router inputs
            xT_r = xT_c[:].rearrange("(ko p) t -> p ko t", p=128)
            gwt_sb = cpool.tile([128, KO_H, E], F32, tag="gwt")
            nc.sync.dma_start(gwt_sb[:], gwt[:].rearrange("(ko p) e -> p ko e", p=128))

            # init idx_dram with OOB sentinels
            oob_sb = cpool.tile([128, GT, 4], I32, tag="oob")
            nc.vector.memset(oob_sb[:], OOB)
            idx_r = idx_dram[:].rearrange("(g p) c -> p g c", p=128)
            init_idx = nc.sync.dma_start(idx_r, oob_sb[:])

            # ======== phase 1: router (fp32) ========
            ag_writes = []
            for tt in range(TI):
                xrot = spool.tile([128, KO_H, 128], F32, tag="xrot", name=f"xr{tt}")
                nc.sync.dma_start(xrot[:], xT_r[:, :, ts(tt, 128)])
                ps_l_full = psA.tile([128, 512], F32, tag="psA", name=f"ps_l{tt}")
                ps_l = ps_l_full[:, :E]
                for k in range(KO_H):
                    nc.tensor.matmul(
                        ps_l,
                        lhsT=xrot[:, k, :],
                        rhs=gwt_sb[:, k, :],
                        start=(k == 0),
                        stop=(k == KO_H - 1),
                    )
                lg_sb = spool.tile([128, E], F32, tag="lgsb")
                nc.scalar.copy(lg_sb[:], ps_l)
                d1 = nc.sync.dma_start(ag_in[ts(tt, 128), :], lg_sb[:])
                nc.sync.dma_start(log_sl[ts(tt, 128), :], lg_sb[:])
                ag_writes.append(d1)

            # ======== phase 2: AllGather of logits ========
            ag = nc.gpsimd.collective_compute(
                kind="AllGather",
                op=mybir.AluOpType.bypass,
                replica_groups=RG,
                ins=[ag_in[:]],
                outs=[ag_out[:]],
            )
            for d in ag_writes:
                add_dep_helper(ag.ins, d.ins, reason="ag after logit writes")

            # ======== phase 3: top-2 + weights + compaction (all T tokens) ========
            rt = cpool.tile([128, TI_ALL, E], F32, tag="rt")
            rt_load = nc.sync.dma_start(
                rt[:], ag_out[:].rearrange("(ti p) e -> p ti e", p=128)
            )
            add_dep_helper(rt_load.ins, ag.ins, reason="rt load after AG")

            max1 = cpool.tile([128, TI_ALL], F32, tag="max1")
            nc.vector.tensor_reduce(max1[:], rt[:], axis=mybir.AxisListType.X,
                                    op=mybir.AluOpType.max)
            eq1 = spool.tile([128, TI_ALL, E], F32, tag="scr8")
            nc.vector.tensor_tensor(eq1[:], rt[:],
                                    max1[:, :, None].to_broadcast([128, TI_ALL, E]),
                                    mybir.AluOpType.is_equal)
            cand = spool.tile([128, TI_ALL, E], F32, tag="scr8")
            nc.vector.tensor_tensor(cand[:], eq1[:],
                                    iota8m[:, None, :].to_broadcast([128, TI_ALL, E]),
                                    mybir.AluOpType.mult)
            nc.vector.tensor_scalar_add(cand[:], cand[:], 8.0)
            idx1 = cpool.tile([128, TI_ALL], F32, tag="idx1")
            nc.vector.tensor_reduce(idx1[:], cand[:], axis=mybir.AxisListType.X,
                                    op=mybir.AluOpType.min)

            # mask out idx1 position by index, find second max
            nmask = spool.tile([128, TI_ALL, E], F32, tag="scr8")
            nc.vector.tensor_tensor(nmask[:],
                                    iota8f[:, None, :].to_broadcast([128, TI_ALL, E]),
                                    idx1[:, :, None].to_broadcast([128, TI_ALL, E]),
                                    mybir.AluOpType.is_equal)
            nc.vector.tensor_scalar_mul(nmask[:], nmask[:], -1e30)
            rt2 = cpool.tile([128, TI_ALL, E], F32, tag="rt2")
            nc.vector.tensor_tensor(rt2[:], rt[:], nmask[:], mybir.AluOpType.add)
            max2 = cpool.tile([128, TI_ALL], F32, tag="max2")
            nc.vector.tensor_reduce(max2[:], rt2[:], axis=mybir.AxisListType.X,
                                    op=mybir.AluOpType.max)
            eq2 = spool.tile([128, TI_ALL, E], F32, tag="scr8")
            nc.vector.tensor_tensor(eq2[:], rt2[:],
                                    max2[:, :, None].to_broadcast([128, TI_ALL, E]),
                                    mybir.AluOpType.is_equal)
            cand2 = spool.tile([128, TI_ALL, E], F32, tag="scr8")
            nc.vector.tensor_tensor(cand2[:], eq2[:],
                                    iota8m[:, None, :].to_broadcast([128, TI_ALL, E]),
                                    mybir.AluOpType.mult)
            nc.vector.tensor_scalar_add(cand2[:], cand2[:], 8.0)
            idx2 = cpool.tile([128, TI_ALL], F32, tag="idx2")
            nc.vector.tensor_reduce(idx2[:], cand2[:], axis=mybir.AxisListType.X,
                                    op=mybir.AluOpType.min)

            # w1 = sigmoid(max1 - max2); my-expert mask + weight
            dmx = cpool.tile([128, TI_ALL], F32, tag="dmx")
            nc.vector.tensor_tensor(dmx[:], max1[:], max2[:], mybir.AluOpType.subtract)
            w1 = cpool.tile([128, TI_ALL], F32, tag="w1")
            nc.scalar.activation(w1[:], dmx[:], mybir.ActivationFunctionType.Sigmoid)

            sel1 = cpool.tile([128, TI_ALL], F32, tag="sel1")
            nc.vector.tensor_tensor(sel1[:], idx1[:],
                                    pidb[:].to_broadcast([128, TI_ALL]),
                                    mybir.AluOpType.is_equal)
            sel2 = cpool.tile([128, TI_ALL], F32, tag="sel2")
            nc.vector.tensor_tensor(sel2[:], idx2[:],
                                    pidb[:].to_broadcast([128, TI_ALL]),
                                    mybir.AluOpType.is_equal)
            m_e = cpool.tile([128, TI_ALL], F32, tag="me")
            nc.vector.tensor_tensor(m_e[:], sel1[:], sel2[:], mybir.AluOpType.add)
            # w_e = sel2 + w1*(sel1-sel2)
            w_e = cpool.tile([128, TI_ALL], F32, tag="we")
            nc.vector.tensor_tensor(w_e[:], sel1[:], sel2[:], mybir.AluOpType.subtract)
            nc.vector.tensor_tensor(w_e[:], w_e[:], w1[:], mybir.AluOpType.mult)
            nc.vector.tensor_tensor(w_e[:], w_e[:], sel2[:], mybir.AluOpType.add)

            # exclusive cumsum of m_e along tokens within each 128-tile (PE)
            ps_c_full = psA.tile([128, 512], F32, tag="psA", name="ps_c")
            ps_c = ps_c_full[:, :TI_ALL]
            nc.tensor.matmul(ps_c, lhsT=ustrict[:], rhs=m_e[:], start=True, stop=True)
            pos = cpool.tile([128, TI_ALL], F32, tag="pos")
            nc.vector.tensor_copy(pos[:], ps_c)

            # per-tile totals S[1, 64] via ones-vector matmul; then exclusive
            # cumsum across tiles (log-shift)
            ones_c = cpool.tile([128, 1], F32, tag="onesc")
            nc.vector.memset(ones_c[:], 1.0)
            ps_s_full = psB.tile([128, 512], F32, tag="psB", name="ps_s")
            ps_s = ps_s_full[:1, :TI_ALL]
            nc.tensor.matmul(ps_s, lhsT=ones_c[:], rhs=m_e[:], start=True, stop=True)
            s_tot = cpool.tile([1, TI_ALL], F32, tag="stot")
            nc.vector.tensor_copy(s_tot[:], ps_s)
            a_t = cpool.tile([1, TI_ALL], F32, tag="csA")
            b_t = cpool.tile([1, TI_ALL], F32, tag="csB")
            nc.vector.tensor_copy(a_t[:], s_tot[:])
            cur, nxt = a_t, b_t
            for sh in [1, 2, 4, 8, 16, 32]:
                nc.vector.tensor_copy(nxt[:, :sh], cur[:, :sh])
                nc.vector.tensor_tensor(nxt[:, sh:], cur[:, sh:], cur[:, : TI_ALL - sh],
                                        mybir.AluOpType.add)
                cur, nxt = nxt, cur
            base_1 = cpool.tile([1, TI_ALL], F32, tag="base1")  # exclusive = incl - S
            nc.vector.tensor_tensor(base_1[:], cur[:], s_tot[:], mybir.AluOpType.subtract)
            base_b = cpool.tile([128, TI_ALL], F32, tag="baseb")
            nc.gpsimd.partition_broadcast(base_b[:], base_1[:])

            dest = cpool.tile([128, TI_ALL], F32, tag="dest")
            nc.vector.tensor_tensor(dest[:], pos[:], base_b[:], mybir.AluOpType.add)
            # unselected tokens -> OOB
            gate_oob = cpool.tile([128, TI_ALL], F32, tag="goob")
            nc.vector.tensor_scalar(gate_oob[:], m_e[:], -1.0e9, 1.0e9,
                                    op0=mybir.AluOpType.mult, op1=mybir.AluOpType.add)
            nc.vector.tensor_tensor(dest[:], dest[:], gate_oob[:], mybir.AluOpType.add)
            dest_i = cpool.tile([128, TI_ALL], I32, tag="desti")
            nc.vector.tensor_copy(dest_i[:], dest[:])

            # payload rows (token_id, w_bits, 0, 0) scattered to idx_dram[dest]
            payload = cpool.tile([128, TI_ALL, 4], I32, tag="payload")
            nc.vector.memset(payload[:], 0)
            nc.vector.tensor_copy(payload[:, :, 0:1], tokid_i[:, :, None])
            nc.vector.tensor_copy(payload[:, :, 1:2].bitcast(F32), w_e[:, :, None])
            scatters = []
            for ti in range(TI_ALL):
                sc = nc.gpsimd.indirect_dma_start(
                    out=idx_dram[:],
                    out_offset=IndirectOffsetOnAxis(ap=dest_i[:, ti : ti + 1], axis=0),
                    in_=payload[:, ti, :],
                    in_offset=None,
                    bounds_check=CAP - 1,
                    oob_is_err=False,
                )
                add_dep_helper(sc.ins, init_idx.ins, reason="scatter after idx init")
                scatters.append(sc)

            # load back the compacted (token_id, weight) table
            idx_sb = cpool.tile([128, GT, 4], I32, tag="idxsb")
            idx_load = nc.sync.dma_start(idx_sb[:], idx_r)
            for sc in scatters:
                add_dep_helper(idx_load.ins, sc.ins, reason="idx load after scatters")

            # weights -> SBUF (bf16)
            wg_sb = wpool.tile([128, KO_H, F], BF16, tag="wg")
            nc.sync.dma_start(wg_sb[:], wg[:].rearrange("(ko p) f -> p ko f", p=128))
            wu_sb = wpool.tile([128, KO_H, F], BF16, tag="wu")
            nc.sync.dma_start(wu_sb[:], wu[:].rearrange("(ko p) f -> p ko f", p=128))
            wd_sb = wpool.tile([128, KO_F, H], BF16, tag="wd")
            nc.sync.dma_start(wd_sb[:], wd[:].rearrange("(ko p) h -> p ko h", p=128))

            # zero the fp16 partial buffer [T, H]
            zero_sb = cpool.tile([128, H], F16, tag="zero")
            nc.vector.memset(zero_sb[:], 0.0)
            part_r = partial[:].rearrange("(t p) h -> p t h", p=128)
            zero_dmas = []
            for j in range(TI_ALL):
                d = nc.sync.dma_start(part_r[:, j, :], zero_sb[:])
                zero_dmas.append(d)


            # ======== phase 4+5: gather + FFN + weighted scatter ========
            rs_deps = list(zero_dmas)
            for m in range(NMEGA):
                xg = gpool.tile([128, 4, H], BF16, tag="xg")
                for j in range(4):
                    g = 4 * m + j
                    nc.gpsimd.indirect_dma_start(
                        out=xg[:, j, :],
                        out_offset=None,
                        in_=x_bf[:],
                        in_offset=IndirectOffsetOnAxis(ap=idx_sb[:, g, 0:1], axis=0),
                        bounds_check=T - 1,
                        oob_is_err=False,
                    )
                # transpose to [H(part), 512]
                xgt = gpool.tile([128, KO_H, 512], BF16, tag="xgt")
                for kt in range(KO_H):
                    ps_t4 = psT.tile([128, 512], BF16, tag="psT", name=f"ps_t{m}_{kt}")
                    for j in range(4):
                        nc.tensor.transpose(ps_t4[:, ts(j, 128)], xg[:, j, ts(kt, 128)], id_bf[:])
                    nc.scalar.copy(xgt[:, kt, :], ps_t4[:])

                # m1/m2: gT[f, tok] = silu(Wg.T x) * (Wu.T x)
                gt_sb = wpool.tile([128, KO_F, 512], BF16, tag="gt")
                for fb in range(KO_F):
                    ps_g = psA.tile([128, 512], F32, tag="psA")
                    ps_u = psB.tile([128, 512], F32, tag="psB")
                    for k in range(KO_H):
                        nc.tensor.matmul(ps_g, lhsT=wg_sb[:, k, ts(fb, 128)],
                                         rhs=xgt[:, k, :],
                                         start=(k == 0), stop=(k == KO_H - 1))
                    for k in range(KO_H):
                        nc.tensor.matmul(ps_u, lhsT=wu_sb[:, k, ts(fb, 128)],
                                         rhs=xgt[:, k, :],
                                         start=(k == 0), stop=(k == KO_H - 1))
                    sil = spool.tile([128, 512], F32, tag="sil")
                    nc.scalar.activation(sil[:], ps_g, mybir.ActivationFunctionType.Silu)
                    nc.vector.tensor_tensor(gt_sb[:, fb, :], sil[:], ps_u,
                                            mybir.AluOpType.mult)

                # m3: out[tok, H] = gT.T @ Wd ; scale by w; scatter to partial
                for tb in range(4):
                    g = 4 * m + tb
                    ps_o = psO.tile([128, H], F32, tag="psO")
                    for fs in range(KO_F):
                        nc.tensor.matmul(ps_o[:, :512],
                                         lhsT=gt_sb[:, fs, ts(tb, 128)],
                                         rhs=wd_sb[:, fs, :512],
                                         start=(fs == 0), stop=(fs == KO_F - 1))
                    for fs in range(KO_F):
                        nc.tensor.matmul(ps_o[:, 512:],
                                         lhsT=gt_sb[:, fs, ts(tb, 128)],
                                         rhs=wd_sb[:, fs, 512:],
                                         start=(fs == 0), stop=(fs == KO_F - 1))
                    outw = spool.tile([128, H], F16, tag="outw")
                    wcol = idx_sb[:, g, 1:2].bitcast(F32)
                    nc.vector.tensor_tensor(outw[:], ps_o[:],
                                            wcol.to_broadcast([128, H]),
                                            mybir.AluOpType.mult)
                    sc = nc.gpsimd.indirect_dma_start(
                        out=partial[:],
                        out_offset=IndirectOffsetOnAxis(ap=idx_sb[:, g, 0:1], axis=0),
                        in_=outw[:],
                        in_offset=None,
                        bounds_check=T - 1,
                        oob_is_err=False,
                    )
                    for z in zero_dmas:
                        add_dep_helper(sc.ins, z.ins, reason="scatter after zeroing")
                    rs_deps.append(sc)

            # ======== phase 6: ReduceScatter(add) ========
            rs = nc.gpsimd.collective_compute(
                kind="ReduceScatter",
                op=mybir.AluOpType.add,
                replica_groups=RG,
                ins=[partial[:]],
                outs=[rs_out[:]],
            )
            for d in rs_deps:
                add_dep_helper(rs.ins, d.ins, reason="rs after partial writes")

            # ======== phase 7: cast fp16 -> fp32 output slice ========
            rs_r = rs_out[:].rearrange("(t p) h -> p t h", p=128)
            out_r = out_sl[:].rearrange("(t p) h -> p t h", p=128)
            for tt in range(TI):
                h16 = spool.tile([128, H], F16, tag="h16")
                ld = nc.sync.dma_start(h16[:], rs_r[:, tt, :])
                add_dep_helper(ld.ins, rs.ins, reason="read rs_out after RS")
                h32 = spool.tile([128, H], F32, tag="h32")
                nc.vector.tensor_copy(h32[:], h16[:])
                nc.sync.dma_start(out_r[:, tt, :], h32[:])

    nc.finalize()
    return nc


def _get_nc():
    if "nc" not in _cached:
        _cached["nc"] = _build()
    return _cached["nc"]


def kernel(hidden_states, gate_w, Wg, Wu, Wd, _trace=False):
    nc = _get_nc()
    b, s, h = hidden_states.shape
    x2d = np.ascontiguousarray(np.asarray(hidden_states, dtype=np.float32).reshape(-1, h))
    gate_w = np.asarray(gate_w, dtype=np.float32)
    x_bf = np.ascontiguousarray(x2d.astype(ml_dtypes.bfloat16))
    gwt = np.ascontiguousarray(gate_w.T)
    Wg = np.asarray(Wg, dtype=np.float32)
    Wu = np.asarray(Wu, dtype=np.float32)
    Wd = np.asarray(Wd, dtype=np.float32)

    in_maps = []
    for c in range(8):
        in_maps.append({
            "xT_c": np.ascontiguousarray(x2d[c * TPC : (c + 1) * TPC].T),
            "x_bf": x_bf,
            "gwt": gwt,
            "wg": np.ascontiguousarray(Wg[c]).astype(ml_dtypes.bfloat16),
            "wu": np.ascontiguousarray(Wu[c]).astype(ml_dtypes.bfloat16),
            "wd": np.ascontiguousarray(Wd[c]).astype(ml_dtypes.bfloat16),
        })

    res = bass_utils.run_bass_kernel_spmd(
        nc, in_maps, core_ids=list(range(8)), trace=_trace
    )
    _cached["last_res"] = res
    out = np.concatenate([r["out_sl"] for r in res.results], axis=0)
    logits = np.concatenate([r["log_sl"] for r in res.results], axis=0)
    return out.reshape(b, s, h).astype(np.float32), logits.astype(np.float32)


# revision 16
# speedup vs baseline: 1.1454x; 1.0016x over previous
"""MoE (LlamaSparseMoeBlock) Trainium2 kernel — expert-parallel over 8 NeuronCores.

Strategy:
  - Core c owns expert c (E == n_cores == 8).
  - Each core computes router logits for its 1/8 token slice (fp32 matmul),
    AllGathers the logits so every core has identical fp32 routing decisions.
  - Each core computes top-2 + normalized weights for ALL tokens, compacts the
    token ids routed to ITS expert (PE prefix-sum matmul + indirect-DMA
    scatter/gather of an index table), gathers those token rows (bf16),
    runs the FFN (bf16 matmuls, fp32 accumulate), scales rows by the routing
    weight, scatters the fp16 rows into a zeroed [T, H] partial buffer, and a
    ReduceScatter(add) combines partials; core c ends with output rows
    [c*1024, (c+1)*1024), cast to fp32.
"""

import numpy as np
import ml_dtypes

import concourse.bass as bass
import concourse.mybir as mybir
from concourse import bacc, bass_utils
from concourse.bass import ts, IndirectOffsetOnAxis
from concourse.tile import TileContext
from concourse.masks import make_identity, make_upper_triangular
from concourse.tile_rust import add_dep_helper

F32 = mybir.dt.float32
F16 = mybir.dt.float16
BF16 = mybir.dt.bfloat16
I32 = mybir.dt.int32
U32 = mybir.dt.uint32

E = 8
K = 2
H = 1024
F = 2048
T = 8192           # total tokens (2*4096)
TPC = T // 8       # tokens per core = 1024
CAP = 2560         # max tokens routed to one expert (20 tiles of 128)
GT = CAP // 128    # gather tiles = 20
NMEGA = GT // 4    # FFN megatiles of 512 tokens = 5
KO_H = H // 128    # 8
KO_F = F // 128    # 16
TI = TPC // 128    # 8 token tiles per core slice
TI_ALL = T // 128  # 64 token tiles globally
RG = [[0, 1, 2, 3, 4, 5, 6, 7]]
OOB = 1_000_000_000

_cached = {}


def _build():
    nc = bacc.Bacc("TRN2", num_devices=8)

    # ---- I/O ----
    xT_c = nc.dram_tensor("xT_c", [H, TPC], F32, kind="ExternalInput")
    x_bf = nc.dram_tensor("x_bf", [T, H], BF16, kind="ExternalInput")
    gwt = nc.dram_tensor("gwt", [H, E], F32, kind="ExternalInput")
    wg = nc.dram_tensor("wg", [H, F], BF16, kind="ExternalInput")
    wu = nc.dram_tensor("wu", [H, F], BF16, kind="ExternalInput")
    wd = nc.dram_tensor("wd", [F, H], BF16, kind="ExternalInput")
    out_sl = nc.dram_tensor("out_sl", [TPC, H], F32, kind="ExternalOutput")
    log_sl = nc.dram_tensor("log_sl", [TPC, E], F32, kind="ExternalOutput")

    # ---- internal DRAM ----
    ag_in = nc.dram_tensor("ag_in", [TPC, E], F32, kind="Internal")
    ag_out = nc.dram_tensor("ag_out", [T, E], F32, kind="Internal", addr_space="Shared")
    idx_dram = nc.dram_tensor("idx_dram", [CAP, 4], I32, kind="Internal")
    partial = nc.dram_tensor("partial", [T, H], F16, kind="Internal")
    rs_out = nc.dram_tensor("rs_out", [TPC, H], F16, kind="Internal")

    with TileContext(nc) as tc:
        with (
            tc.tile_pool(name="wpool", bufs=1) as wpool,
            tc.tile_pool(name="cpool", bufs=1) as cpool,
            tc.tile_pool(name="spool", bufs=2) as spool,
            tc.tile_pool(name="gpool", bufs=2) as gpool,
            tc.tile_pool(name="psA", bufs=1, space="PSUM") as psA,
            tc.tile_pool(name="psB", bufs=1, space="PSUM") as psB,
            tc.tile_pool(name="psT", bufs=2, space="PSUM") as psT,
            tc.tile_pool(name="psO", bufs=2, space="PSUM") as psO,
        ):
            # ======== phase 0: constants + preloads ========
            id_bf = cpool.tile([128, 128], BF16, tag="id_bf")
            make_identity(nc, id_bf[:])
            ustrict = cpool.tile([128, 128], F32, tag="ustrict")
            make_upper_triangular(nc, ustrict[:], val=1.0, diag=False)

            iota8_i = cpool.tile([128, E], I32, tag="iota8i")
            nc.gpsimd.iota(iota8_i[:], pattern=[[1, E]], base=0, channel_multiplier=0)
            iota8f = cpool.tile([128, E], F32, tag="iota8f")
            nc.vector.tensor_copy(iota8f[:], iota8_i[:])
            iota8m = cpool.tile([128, E], F32, tag="iota8m")  # iota - 8
            nc.vector.tensor_scalar_sub(iota8m[:], iota8f[:], 8.0)

            tokid_i = cpool.tile([128, TI_ALL], I32, tag="tokidi")
            nc.gpsimd.iota(tokid_i[:], pattern=[[128, TI_ALL]], base=0, channel_multiplier=1)

            # partition id -> fp32 broadcast [128, 1]
            pid_u = cpool.tile([1, 1], U32, tag="pidu")
            assert nc.partition_id_tensor is not None
            nc.sync.dma_start(pid_u[:], nc.partition_id_tensor[:])
            pid_f1 = cpool.tile([1, 1], F32, tag="pidf1")
            nc.vector.tensor_copy(pid_f1[:], pid_u[:])
            pidb = cpool.tile([128, 1], F32, tag="pidb")
            nc.gpsimd.partition_broadcast(pidb[:], pid_f1[:])

            # router inputs
            xT_r = xT_c[:].rearrange("(ko p) t -> p ko t", p=128)
            gwt_sb = cpool.tile([128, KO_H, E], F32, tag="gwt")
            nc.sync.dma_start(gwt_sb[:], gwt[:].rearrange("(ko p) e -> p ko e", p=128))

            # init idx_dram with OOB sentinels
            oob_sb = cpool.tile([128, GT, 4], I32, tag="oob")
            nc.vector.memset(oob_sb[:], OOB)
            idx_r = idx_dram[:].rearrange("(g p) c -> p g c", p=128)
            init_idx = nc.sync.dma_start(idx_r, oob_sb[:])

            # ======== phase 1: router (fp32) ========
            ag_writes = []
            for tt in range(TI):
                xrot = spool.tile([128, KO_H, 128], F32, tag="xrot", name=f"xr{tt}")
                nc.sync.dma_start(xrot[:], xT_r[:, :, ts(tt, 128)])
                ps_l_full = psA.tile([128, 512], F32, tag="psA", name=f"ps_l{tt}")
                ps_l = ps_l_full[:, :E]
                for k in range(KO_H):
                    nc.tensor.matmul(
                        ps_l,
                        lhsT=xrot[:, k, :],
                        rhs=gwt_sb[:, k, :],
                        start=(k == 0),
                        stop=(k == KO_H - 1),
                    )
                lg_sb = spool.tile([128, E], F32, tag="lgsb")
                nc.scalar.copy(lg_sb[:], ps_l)
                d1 = nc.sync.dma_start(ag_in[ts(tt, 128), :], lg_sb[:])
                nc.sync.dma_start(log_sl[ts(tt, 128), :], lg_sb[:])
                ag_writes.append(d1)

            # ======== phase 2: AllGather of logits ========
            ag = nc.gpsimd.collective_compute(
                kind="AllGather",
                op=mybir.AluOpType.bypass,
                replica_groups=RG,
                ins=[ag_in[:]],
                outs=[ag_out[:]],
            )
            for d in ag_writes:
                add_dep_helper(ag.ins, d.ins, reason="ag after logit writes")

            # ======== phase 3: top-2 + weights + compaction (all T tokens) ========
            rt = cpool.tile([128, TI_ALL, E], F32, tag="rt")
            rt_load = nc.sync.dma_start(
                rt[:], ag_out[:].rearrange("(ti p) e -> p ti e", p=128)
            )
            add_dep_helper(rt_load.ins, ag.ins, reason="rt load after AG")

            max1 = cpool.tile([128, TI_ALL], F32, tag="max1")
            nc.vector.tensor_reduce(max1[:], rt[:], axis=mybir.AxisListType.X,
                                    op=mybir.AluOpType.max)
            eq1 = spool.tile([128, TI_ALL, E], F32, tag="scr8")
            nc.vector.tensor_tensor(eq1[:], rt[:],
                                    max1[:, :, None].to_broadcast([128, TI_ALL, E]),
                                    mybir.AluOpType.is_equal)
            cand = spool.tile([128, TI_ALL, E], F32, tag="scr8")
            nc.vector.tensor_tensor(cand[:], eq1[:],
                                    iota8m[:, None, :].to_broadcast([128, TI_ALL, E]),
                                    mybir.AluOpType.mult)
            nc.vector.tensor_scalar_add(cand[:], cand[:], 8.0)
            idx1 = cpool.tile([128, TI_ALL], F32, tag="idx1")
            nc.vector.tensor_reduce(idx1[:], cand[:], axis=mybir.AxisListType.X,
                                    op=mybir.AluOpType.min)

            # mask out idx1 position by index, find second max
            nmask = spool.tile([128, TI_ALL, E], F32, tag="scr8")
            nc.vector.tensor_tensor(nmask[:],
                                    iota8f[:, None, :].to_broadcast([128, TI_ALL, E]),
                                    idx1[:, :, None].to_broadcast([128, TI_ALL, E]),
                                    mybir.AluOpType.is_equal)
            nc.vector.tensor_scalar_mul(nmask[:], nmask[:], -1e30)
            rt2 = cpool.tile([128, TI_ALL, E], F32, tag="rt2")
            nc.vector.tensor_tensor(rt2[:], rt[:], nmask[:], mybir.AluOpType.add)
            max2 = cpool.tile([128, TI_ALL], F32, tag="max2")
            nc.vector.tensor_reduce(max2[:], rt2[:], axis=mybir.AxisListType.X,
                                    op=mybir.AluOpType.max)
            eq2 = spool.tile([128, TI_ALL, E], F32, tag="scr8")
            nc.vector.tensor_tensor(eq2[:], rt2[:],
                                    max2[:, :, None].to_broadcast([128, TI_ALL, E]),
                                    mybir.AluOpType.is_equal)
            cand2 = spool.tile([128, TI_ALL, E], F32, tag="scr8")
            nc.vector.tensor_tensor(cand2[:], eq2[:],
                                    iota8m[:, None, :].to_broadcast([128, TI_ALL, E]),
                                    mybir.AluOpType.mult)
            nc.vector.tensor_scalar_add(cand2[:], cand2[:], 8.0)
            idx2 = cpool.tile([128, TI_ALL], F32, tag="idx2")
            nc.vector.tensor_reduce(idx2[:], cand2[:], axis=mybir.AxisListType.X,
                                    op=mybir.AluOpType.min)

            # w1 = sigmoid(max1 - max2); my-expert mask + weight
            dmx = cpool.tile([128, TI_ALL], F32, tag="dmx")
            nc.vector.tensor_tensor(dmx[:], max1[:], max2[:], mybir.AluOpType.subtract)
            w1 = cpool.tile([128, TI_ALL], F32, tag="w1")
            nc.scalar.activation(w1[:], dmx[:], mybir.ActivationFunctionType.Sigmoid)

            sel1 = cpool.tile([128, TI_ALL], F32, tag="sel1")
            nc.vector.tensor_tensor(sel1[:], idx1[:],
                                    pidb[:].to_broadcast([128, TI_ALL]),
                                    mybir.AluOpType.is_equal)
            sel2 = cpool.tile([128, TI_ALL], F32, tag="sel2")
            nc.vector.tensor_tensor(sel2[:], idx2[:],
                                    pidb[:].to_broadcast([128, TI_ALL]),
                                    mybir.AluOpType.is_equal)
            m_e = cpool.tile([128, TI_ALL], F32, tag="me")
            nc.vector.tensor_tensor(m_e[:], sel1[:], sel2[:], mybir.AluOpType.add)
            # w_e = sel2 + w1*(sel1-sel2)
            w_e = cpool.tile([128, TI_ALL], F32, tag="we")
            nc.vector.tensor_tensor(w_e[:], sel1[:], sel2[:], mybir.AluOpType.subtract)
            nc.vector.tensor_tensor(w_e[:], w_e[:], w1[:], mybir.AluOpType.mult)
            nc.vector.tensor_tensor(w_e[:], w_e[:], sel2[:], mybir.AluOpType.add)

            # exclusive cumsum of m_e along tokens within each 128-tile (PE)
            ps_c_full = psA.tile([128, 512], F32, tag="psA", name="ps_c")
            ps_c = ps_c_full[:, :TI_ALL]
            nc.tensor.matmul(ps_c, lhsT=ustrict[:], rhs=m_e[:], start=True, stop=True)
            pos = cpool.tile([128, TI_ALL], F32, tag="pos")
            nc.vector.tensor_copy(pos[:], ps_c)

            # per-tile totals S[1, 64] via ones-vector matmul; then exclusive
            # cumsum across tiles (log-shift)
            ones_c = cpool.tile([128, 1], F32, tag="onesc")
            nc.vector.memset(ones_c[:], 1.0)
            ps_s_full = psB.tile([128, 512], F32, tag="psB", name="ps_s")
            ps_s = ps_s_full[:1, :TI_ALL]
            nc.tensor.matmul(ps_s, lhsT=ones_c[:], rhs=m_e[:], start=True, stop=True)
            s_tot = cpool.tile([1, TI_ALL], F32, tag="stot")
            nc.vector.tensor_copy(s_tot[:], ps_s)
            a_t = cpool.tile([1, TI_ALL], F32, tag="csA")
            b_t = cpool.tile([1, TI_ALL], F32, tag="csB")
            nc.vector.tensor_copy(a_t[:], s_tot[:])
            cur, nxt = a_t, b_t
            for sh in [1, 2, 4, 8, 16, 32]:
                nc.vector.tensor_copy(nxt[:, :sh], cur[:, :sh])
                nc.vector.tensor_tensor(nxt[:, sh:], cur[:, sh:], cur[:, : TI_ALL - sh],
                                        mybir.AluOpType.add)
                cur, nxt = nxt, cur
            base_1 = cpool.tile([1, TI_ALL], F32, tag="base1")  # exclusive = incl - S
            nc.vector.tensor_tensor(base_1[:], cur[:], s_tot[:], mybir.AluOpType.subtract)
            base_b = cpool.tile([128, TI_ALL], F32, tag="baseb")
            nc.gpsimd.partition_broadcast(base_b[:], base_1[:])

            dest = cpool.tile([128, TI_ALL], F32, tag="dest")
            nc.vector.tensor_tensor(dest[:], pos[:], base_b[:], mybir.AluOpType.add)
            # unselected tokens -> OOB
            gate_oob = cpool.tile([128, TI_ALL], F32, tag="goob")
            nc.vector.tensor_scalar(gate_oob[:], m_e[:], -1.0e9, 1.0e9,
                                    op0=mybir.AluOpType.mult, op1=mybir.AluOpType.add)
            nc.vector.tensor_tensor(dest[:], dest[:], gate_oob[:], mybir.AluOpType.add)
            dest_i = cpool.tile([128, TI_ALL], I32, tag="desti")
            nc.vector.tensor_copy(dest_i[:], dest[:])

            # payload rows (token_id, w_bits, 0, 0) scattered to idx_dram[dest]
            payload = cpool.tile([128, TI_ALL, 4], I32, tag="payload")
            nc.vector.memset(payload[:], 0)
            nc.vector.tensor_copy(payload[:, :, 0:1], tokid_i[:, :, None])
            nc.vector.tensor_copy(payload[:, :, 1:2].bitcast(F32), w_e[:, :, None])
            scatters = []
            for ti in range(TI_ALL):
                sc = nc.gpsimd.indirect_dma_start(
                    out=idx_dram[:],
                    out_offset=IndirectOffsetOnAxis(ap=dest_i[:, ti : ti + 1], axis=0),
                    in_=payload[:, ti, :],
                    in_offset=None,
                    bounds_check=CAP - 1,
                    oob_is_err=False,
                )
                add_dep_helper(sc.ins, init_idx.ins, reason="scatter after idx init")
                scatters.append(sc)

            # load back the compacted (token_id, weight) table
            idx_sb = cpool.tile([128, GT, 4], I32, tag="idxsb")
            idx_load = nc.sync.dma_start(idx_sb[:], idx_r)
            for sc in scatters:
                add_dep_helper(idx_load.ins, sc.ins, reason="idx load after scatters")

            # weights -> SBUF (bf16)
            wg_sb = wpool.tile([128, KO_H, F], BF16, tag="wg")
            nc.sync.dma_start(wg_sb[:], wg[:].rearrange("(ko p) f -> p ko f", p=128))
            wu_sb = wpool.tile([128, KO_H, F], BF16, tag="wu")
            nc.sync.dma_start(wu_sb[:], wu[:].rearrange("(ko p) f -> p ko f", p=128))
            wd_sb = wpool.tile([128, KO_F, H], BF16, tag="wd")
            nc.sync.dma_start(wd_sb[:], wd[:].rearrange("(ko p) h -> p ko h", p=128))

            # zero the fp16 partial buffer [T, H]
            zero_sb = cpool.tile([128, H], F16, tag="zero")
            nc.vector.memset(zero_sb[:], 0.0)
            part_r = partial[:].rearrange("(t p) h -> p t h", p=128)
            zero_dmas = []
            for j in range(TI_ALL):
                d = nc.sync.dma_start(part_r[:, j, :], zero_sb[:])
                zero_dmas.append(d)


            # ======== phase 4+5: gather + FFN + weighted scatter ========
            rs_deps = list(zero_dmas)
            for m in range(NMEGA):
                xg = gpool.tile([128, 4, H], BF16, tag="xg")
                for j in range(4):
                    g = 4 * m + j
                    nc.gpsimd.indirect_dma_start(
                        out=xg[:, j, :],
                        out_offset=None,
                        in_=x_bf[:],
                        in_offset=IndirectOffsetOnAxis(ap=idx_sb[:, g, 0:1], axis=0),
                        bounds_check=T - 1,
                        oob_is_err=False,
                    )
                # transpose to [H(part), 512]
                xgt = gpool.tile([128, KO_H, 512], BF16, tag="xgt")
                for kt in range(KO_H):
                    ps_t4 = psT.tile([128, 512], BF16, tag="psT", name=f"ps_t{m}_{kt}")
                    for j in range(4):
                        nc.tensor.transpose(ps_t4[:, ts(j, 128)], xg[:, j, ts(kt, 128)], id_bf[:])
                    nc.scalar.copy(xgt[:, kt, :], ps_t4[:])

                # m1/m2: gT[f, tok] = silu(Wg.T x) * (Wu.T x)
                gt_sb = wpool.tile([128, KO_F, 512], BF16, tag="gt")
                for fb in range(KO_F):
                    ps_g = psA.tile([128, 512], F32, tag="psA")
                    ps_u = psB.tile([128, 512], F32, tag="psB")
                    for k in range(KO_H):
                        nc.tensor.matmul(ps_g, lhsT=wg_sb[:, k, ts(fb, 128)],
                                         rhs=xgt[:, k, :],
                                         start=(k == 0), stop=(k == KO_H - 1))
                    for k in range(KO_H):
                        nc.tensor.matmul(ps_u, lhsT=wu_sb[:, k, ts(fb, 128)],
                                         rhs=xgt[:, k, :],
                                         start=(k == 0), stop=(k == KO_H - 1))
                    sil = spool.tile([128, 512], F32, tag="sil")
                    nc.scalar.activation(sil[:], ps_g, mybir.ActivationFunctionType.Silu)
                    nc.vector.tensor_tensor(gt_sb[:, fb, :], sil[:], ps_u,
                                            mybir.AluOpType.mult)

                # m3: out[tok, H] = gT.T @ Wd ; scale by w; scatter to partial
                for tb in range(4):
                    g = 4 * m + tb
                    ps_o = psO.tile([128, H], F32, tag="psO")
                    for fs in range(KO_F):
                        nc.tensor.matmul(ps_o[:, :512],
                                         lhsT=gt_sb[:, fs, ts(tb, 128)],
                                         rhs=wd_sb[:, fs, :512],
                                         start=(fs == 0), stop=(fs == KO_F - 1))
                    for fs in range(KO_F):
                        nc.tensor.matmul(ps_o[:, 512:],
                                         lhsT=gt_sb[:, fs, ts(tb, 128)],
                                         rhs=wd_sb[:, fs, 512:],
                                         start=(fs == 0), stop=(fs == KO_F - 1))
                    outw = spool.tile([128, H], F16, tag="outw")
                    wcol = idx_sb[:, g, 1:2].bitcast(F32)
                    nc.vector.tensor_tensor(outw[:], ps_o[:],
                                            wcol.to_broadcast([128, H]),
                                            mybir.AluOpType.mult)
                    sc = nc.gpsimd.indirect_dma_start(
                        out=partial[:],
                        out_offset=IndirectOffsetOnAxis(ap=idx_sb[:, g, 0:1], axis=0),
                        in_=outw[:],
                        in_offset=None,
                        bounds_check=T - 1,
                        oob_is_err=False,
                    )
                    for z in zero_dmas:
                        add_dep_helper(sc.ins, z.ins, reason="scatter after zeroing")
                    rs_deps.append(sc)

            # ======== phase 6: ReduceScatter(add) ========
            rs = nc.gpsimd.collective_compute(
                kind="ReduceScatter",
                op=mybir.AluOpType.add,
                replica_groups=RG,
                ins=[partial[:]],
                outs=[rs_out[:]],
            )
            for d in rs_deps:
                add_dep_helper(rs.ins, d.ins, reason="rs after partial writes")

            # ======== phase 7: cast fp16 -> fp32 output slice ========
            rs_r = rs_out[:].rearrange("(t p) h -> p t h", p=128)
            out_r = out_sl[:].rearrange("(t p) h -> p t h", p=128)
            for tt in range(TI):
                h16 = spool.tile([128, H], F16, tag="h16")
                ld = nc.sync.dma_start(h16[:], rs_r[:, tt, :])
                add_dep_helper(ld.ins, rs.ins, reason="read rs_out after RS")
                h32 = spool.tile([128, H], F32, tag="h32")
                nc.vector.tensor_copy(h32[:], h16[:])
                nc.sync.dma_start(out_r[:, tt, :], h32[:])

    nc.finalize()
    return nc


def _get_nc():
    if "nc" not in _cached:
        _cached["nc"] = _build()
    return _cached["nc"]


def kernel(hidden_states, gate_w, Wg, Wu, Wd, _trace=False):
    nc = _get_nc()
    b, s, h = hidden_states.shape
    x2d = np.ascontiguousarray(np.asarray(hidden_states, dtype=np.float32).reshape(-1, h))
    gate_w = np.asarray(gate_w, dtype=np.float32)
    x_bf = np.ascontiguousarray(x2d.astype(ml_dtypes.bfloat16))
    gwt = np.ascontiguousarray(gate_w.T)
    Wg = np.asarray(Wg, dtype=np.float32)
    Wu = np.asarray(Wu, dtype=np.float32)
    Wd = np.asarray(Wd, dtype=np.float32)

    in_maps = []
    for c in range(8):
        in_maps.append({
            "xT_c": np.ascontiguousarray(x2d[c * TPC : (c + 1) * TPC].T),
            "x_bf": x_bf,
            "gwt": gwt,
            "wg": np.ascontiguousarray(Wg[c]).astype(ml_dtypes.bfloat16),
            "wu": np.ascontiguousarray(Wu[c]).astype(ml_dtypes.bfloat16),
            "wd": np.ascontiguousarray(Wd[c]).astype(ml_dtypes.bfloat16),
        })

    res = bass_utils.run_bass_kernel_spmd(
        nc, in_maps, core_ids=list(range(8)), trace=_trace
    )
    _cached["last_res"] = res
    out = np.concatenate([r["out_sl"] for r in res.results], axis=0)
    logits = np.concatenate([r["log_sl"] for r in res.results], axis=0)
    return out.reshape(b, s, h).astype(np.float32), logits.astype(np.float32)
